# revision 3
# baseline (speedup 1.0000x reference)
"""Trainium2 Bass kernel for the dual channel-attention module.

Data-parallel over batch: B=8 -> one batch item per NeuronCore. Each core runs
two independent pipelines (y -> o1, x -> o2); each pipeline is:
  3x3 conv projections (Q,K stride 2; V stride 1) fused with BatchNorm,
  channel attention S = Q K^T (over tokens), softmax over channels,
  ctx = mean_h softmax(S) @ V, out = ctx^T @ W_out^T.

All matmuls run as float32r (full PE rate at free-dim>=256, fp22 mantissa).
BN scale (and the attention 1/sqrt(C) for Q, and the 1/heads for the output
projection) are folded into weights on the host; BN bias is applied via a
ones-column bias matmul (Q/K, channel on free axis) or per-partition
activation bias (V, channel on partition axis).
"""

import os
import sys

sys.path.insert(0, '/opt/trn_rl_repo')

import numpy as np

import concourse.bacc as bacc
import concourse.mybir as mybir
import concourse.tile as tile
from concourse.bass_utils import run_bass_kernel_spmd
from concourse.masks import make_identity

F32 = mybir.dt.float32
F32R = mybir.dt.float32r
AF = mybir.ActivationFunctionType
AX = mybir.AxisListType

P = 128
C = 256          # channels
HEADS = 4
NCORES = 8
EPS = 1e-5

_programs = {}


def _build_program(H, W):
    """One-core program; same NEFF runs SPMD on all 8 cores."""
    N = H * W                 # stride-1 token count
    PH, PW = H + 2, W + 2     # padded image dims
    OH, OW = H // 2, W // 2   # stride-2 output dims
    NQ = OH * OW              # stride-2 token count
    T = NQ // P               # q/k token chunks
    RQ = P // OW              # stride-2 output rows per token chunk
    T2 = N // P               # input token chunks (and proj chunks)
    NT = N // 512             # v-conv tiles of 512 tokens
    RPN = 512 // W            # image rows per v tile
    CC = C // P               # channel chunks (2)

    nc = bacc.Bacc("TRN2", target_bir_lowering=False, debug=False,
                   num_devices=NCORES)

    # ---- I/O ----
    xin = [nc.dram_tensor(f"in{s}", [N, C], F32, kind="ExternalInput").ap()
           for s in range(2)]
    wqk = nc.dram_tensor("wqk", [2, 2, HEADS, CC, P, 9, C], F32R,
                         kind="ExternalInput").ap()
    wv = nc.dram_tensor("wv", [2, HEADS, CC, P, 9, C], F32R,
                        kind="ExternalInput").ap()
    bqk = nc.dram_tensor("bqk", [2, 2, P, HEADS, C], F32R,
                         kind="ExternalInput").ap()
    bv = nc.dram_tensor("bv", [2, HEADS, CC, P], F32,
                        kind="ExternalInput").ap()
    wo = nc.dram_tensor("wo", [2, CC, P, C], F32R, kind="ExternalInput").ap()
    outs = [nc.dram_tensor(f"out{s}", [N, C], F32, kind="ExternalOutput").ap()
            for s in range(2)]

    # tap decomposition for stride-2 grids: (dy,dx) -> grid (py,px,b) + row off a
    # grid combos (py, px, b): 6 of them
    combos = [(0, 0, 0), (0, 1, 0), (0, 0, 1), (1, 0, 0), (1, 1, 0), (1, 0, 1)]
    combo_idx = {c: i for i, c in enumerate(combos)}

    with tile.TileContext(nc) as tc:
        import contextlib
        est = contextlib.ExitStack()
        with est:
            consts = est.enter_context(tc.tile_pool(name="consts", bufs=1))
            ps_tr = est.enter_context(
                tc.tile_pool(name="ps_tr", bufs=4, space="PSUM"))
            ps_ctx = est.enter_context(
                tc.tile_pool(name="ps_ctx", bufs=4, space="PSUM"))

            ident = consts.tile([P, P], F32)
            make_identity(nc, ident[:])
            ones_f = consts.tile([P, P], F32)
            nc.vector.memset(ones_f[:], 1.0)
            ones = consts.tile([P, P], F32R)
            nc.vector.tensor_copy(ones[:], ones_f[:])
            zeros_f = consts.tile([P, 2 * PW], F32)
            nc.vector.memset(zeros_f[:], 0.0)
            ident_r = consts.tile([P, P], F32R)
            nc.vector.tensor_copy(ident_r[:], ident[:])

            for s in range(2):
                with contextlib.ExitStack() as sst:
                    sb_img = sst.enter_context(
                        tc.tile_pool(name=f"img{s}", bufs=1))
                    sb_keep = sst.enter_context(
                        tc.tile_pool(name=f"keep{s}", bufs=1))
                    sb_work = sst.enter_context(
                        tc.tile_pool(name=f"work{s}", bufs=1))

                    # ---------- Phase A: padded channel-major image ----------
                    img = [sb_img.tile([P, PH, PW], F32R, name=f"imgc{cc}",
                                       tag=f"imgc{cc}") for cc in range(CC)]
                    for cc in range(CC):
                        # zero borders: top+bottom rows, then left+right cols
                        nc.vector.tensor_copy(
                            img[cc][:, 0:PH:PH - 1, :], zeros_f[:, : 2 * PW]
                            .rearrange("p (a b) -> p a b", a=2))
                        nc.vector.tensor_copy(
                            img[cc][:, 1:PH - 1, 0:PW:PW - 1],
                            zeros_f[:, : 2 * H]
                            .rearrange("p (a b) -> p b a", a=2))
                    for t in range(T2):
                        tok = sb_work.tile([P, C], F32, name="tok", tag="tok",
                                           bufs=3)
                        nc.sync.dma_start(tok[:], xin[s][t * P:(t + 1) * P, :])
                        r0 = (t * P) // W          # image row of first token
                        nr = P // W                # rows per chunk
                        for cc in range(CC):
                            ptp = ps_tr.tile([P, P], F32, name="ptp", tag="pst")
                            nc.tensor.transpose(
                                ptp[:], tok[:, cc * P:(cc + 1) * P], ident[:])
                            nc.vector.tensor_copy(
                                img[cc][:, 1 + r0:1 + r0 + nr, 1:1 + W],
                                ptp[:].rearrange("p (a b) -> p a b", a=nr))

                    # ---------- Phase B: parity grids for stride-2 convs ----
                    gr = [[sb_keep.tile([P, (OH + 1) * OW], F32R,
                                        name=f"g{gi}_{cc}", tag=f"g{gi}_{cc}")
                           for cc in range(CC)] for gi in range(6)]
                    for gi, (py, px, b) in enumerate(combos):
                        c0 = 2 * b + px
                        for cc in range(CC):
                            nc.vector.tensor_copy(
                                gr[gi][cc][:].rearrange(
                                    "p (u v) -> p u v", u=OH + 1),
                                img[cc][:, py: py + 2 * OH + 1: 2,
                                        c0: c0 + 2 * OW - 1: 2])

                    # ---------- Phase C: Q/K convs + attention ----------
                    with contextlib.ExitStack() as cst:
                        sb_qkw = cst.enter_context(
                            tc.tile_pool(name=f"qkw{s}", bufs=1))
                        sb_qk = cst.enter_context(
                            tc.tile_pool(name=f"qk{s}", bufs=1))

                        biasb = [sb_qk.tile([P, HEADS, C], F32R,
                                            name=f"biasb{qk}", tag=f"biasb{qk}")
                                 for qk in range(2)]
                        for qk in range(2):
                            nc.sync.dma_start(biasb[qk][:], bqk[s, qk])

                        pT = [sb_keep.tile([P, CC, C], F32R, name=f"pT{h}",
                                           tag=f"pT{h}") for h in range(HEADS)]

                        for h in range(HEADS):
                            qtm = []
                            ktm = []
                            for qk in range(2):
                                dst = []
                                for t in range(T):
                                    dst.append(sb_qk.tile(
                                        [P, C], F32R, name=f"qk{qk}t{t}",
                                        tag=f"qk{qk}t{t}"))
                                (qtm if qk == 0 else ktm).extend(dst)
                                wt = [sb_qkw.tile([P, 9, C], F32R,
                                                  name=f"wqk{qk}c{ci}",
                                                  tag="qkw", bufs=5)
                                      for ci in range(CC)]
                                for ci in range(CC):
                                    nc.sync.dma_start(
                                        wt[ci][:], wqk[s, qk, h, ci])
                                for t in range(T):
                                    acc = ps_tr.tile([P, C], F32, name="qkacc",
                                                     tag="pst")
                                    first = True
                                    for ci in range(CC):
                                        for tap in range(9):
                                            dy, dx = tap // 3, tap % 3
                                            gi = combo_idx[
                                                (dy & 1, dx & 1, dx >> 1)]
                                            a = dy >> 1
                                            off = (t * RQ + a) * OW
                                            nc.tensor.matmul(
                                                acc[:],
                                                gr[gi][ci][:, off:off + P],
                                                wt[ci][:, tap, :],
                                                start=first, stop=False)
                                            first = False
                                    nc.tensor.matmul(
                                        acc[:], ones[:, :P], biasb[qk][:, h, :],
                                        start=False, stop=True)
                                    nc.scalar.copy(dst[t][:], acc[:])

                            # S = Q^T K (contract tokens), softmax over free axis
                            for ccb in range(CC):
                                sacc = ps_tr.tile([P, C], F32, name="sacc",
                                                  tag="pst")
                                for t in range(T):
                                    nc.tensor.matmul(
                                        sacc[:],
                                        qtm[t][:, ccb * P:(ccb + 1) * P],
                                        ktm[t][:],
                                        start=(t == 0), stop=(t == T - 1))
                                negmax = sb_work.tile([P, 1], F32,
                                                      name="negmax",
                                                      tag="negmax", bufs=2)
                                nc.vector.reduce_max(negmax[:], sacc[:],
                                                     axis=AX.X, negate=True)
                                e = sb_work.tile([P, C], F32, name="esm",
                                                 tag="esm", bufs=2)
                                esum = sb_work.tile([P, 1], F32, name="esum",
                                                    tag="esum", bufs=2)
                                nc.scalar.activation(e[:], sacc[:], AF.Exp,
                                                     bias=negmax[:], scale=1.0,
                                                     accum_out=esum[:])
                                rec = sb_work.tile([P, 1], F32, name="rec",
                                                   tag="rec", bufs=2)
                                nc.vector.reciprocal(rec[:], esum[:])
                                pn = sb_work.tile([P, C], F32, name="pn",
                                                  tag="pn", bufs=2)
                                nc.vector.tensor_scalar_mul(pn[:], e[:], rec[:])
                                for dc in range(CC):
                                    ptp = ps_tr.tile([P, P], F32, name="ptp2",
                                                     tag="pst")
                                    nc.tensor.transpose(
                                        ptp[:], pn[:, dc * P:(dc + 1) * P],
                                        ident[:])
                                    nc.vector.tensor_copy(
                                        pT[h][:, dc, ccb * P:(ccb + 1) * P],
                                        ptp[:])

                    # ---------- Phase D: V convs + context ----------
                    ctx = [sb_keep.tile([P, N], F32R, name=f"ctx{ccb}",
                                        tag=f"ctx{ccb}") for ccb in range(CC)]
                    with contextlib.ExitStack() as dst_:
                        sb_vw = dst_.enter_context(
                            tc.tile_pool(name=f"vw{s}", bufs=1))
                        for h in range(HEADS):
                            wt = [sb_vw.tile([P, 9, C], F32R,
                                             name=f"wv{ci}", tag="vw", bufs=5)
                                  for ci in range(CC)]
                            for ci in range(CC):
                                nc.sync.dma_start(wt[ci][:], wv[s, h, ci])
                            bvt = [sb_vw.tile([P, 1], F32, name=f"bv{dc}",
                                              tag=f"bvt{dc}", bufs=2)
                                   for dc in range(CC)]
                            for dc in range(CC):
                                nc.sync.dma_start(
                                    bvt[dc][:],
                                    bv[s, h, dc].unsqueeze(1))
                            for nt in range(NT):
                                vsb = []
                                r0 = nt * RPN
                                for dc in range(CC):
                                    vacc = ps_tr.tile([P, 512], F32,
                                                      name="vacc", tag="pst")
                                    first = True
                                    for ci in range(CC):
                                        for tap in range(9):
                                            dy, dx = tap // 3, tap % 3
                                            nc.tensor.matmul(
                                                vacc[:],
                                                wt[ci][:, tap,
                                                       dc * P:(dc + 1) * P],
                                                img[ci][:, r0 + dy:
                                                        r0 + dy + RPN,
                                                        dx:dx + W],
                                                start=first,
                                                stop=(ci == CC - 1 and
                                                      tap == 8))
                                            first = False
                                    vt = sb_work.tile([P, 512], F32R,
                                                      name="vsb", tag="vsb",
                                                      bufs=4)
                                    nc.scalar.activation(vt[:], vacc[:],
                                                         AF.Identity,
                                                         bias=bvt[dc][:],
                                                         scale=1.0)
                                    vsb.append(vt)
                                for ccb in range(CC):
                                    cacc = ps_ctx.tile([P, 512], F32,
                                                       name="cacc", tag="psc")
                                    for dc in range(CC):
                                        nc.tensor.matmul(
                                            cacc[:],
                                            pT[h][:, dc,
                                                  ccb * P:(ccb + 1) * P],
                                            vsb[dc][:],
                                            start=(dc == 0),
                                            stop=(dc == CC - 1))
                                    dst_ap = ctx[ccb][:, nt * 512:
                                                      (nt + 1) * 512]
                                    if h == 0:
                                        nc.vector.tensor_copy(dst_ap, cacc[:])
                                    else:
                                        nc.vector.tensor_add(
                                            out=dst_ap, in0=dst_ap,
                                            in1=cacc[:])

                    # ---------- Phase E: output projection ----------
                    wot = [sb_work.tile([P, C], F32R, name=f"wo{ccb}",
                                        tag=f"wo{ccb}") for ccb in range(CC)]
                    for ccb in range(CC):
                        nc.sync.dma_start(wot[ccb][:], wo[s, ccb])
                    for t in range(T2):
                        oacc = ps_tr.tile([P, C], F32, name="oacc", tag="pst")
                        for ccb in range(CC):
                            nc.tensor.matmul(
                                oacc[:],
                                ctx[ccb][:, t * P:(t + 1) * P],
                                wot[ccb][:],
                                start=(ccb == 0), stop=(ccb == CC - 1))
                        osb = sb_work.tile([P, C], F32, name="osb", tag="osb",
                                           bufs=3)
                        nc.scalar.copy(osb[:], oacc[:])
                        nc.sync.dma_start(outs[s][t * P:(t + 1) * P, :],
                                          osb[:])

    nc.compile()
    return nc


def _prep_weights(w_conv, bn_gamma, bn_beta, bn_mean, bn_var, w_out1, w_out2):
    """Fold BN into conv weights/biases and pack into kernel layouts."""
    w_conv = np.asarray(w_conv, np.float32)
    scale = np.asarray(bn_gamma, np.float32) / np.sqrt(
        np.asarray(bn_var, np.float32) + EPS)            # [6,4,256]
    shift = np.asarray(bn_beta, np.float32) - np.asarray(
        bn_mean, np.float32) * scale

    wf = w_conv * scale[:, :, :, None, None, None]       # [6,4,co,ci,3,3]
    sa = 1.0 / np.sqrt(C)
    wf[0] *= sa
    wf[1] *= sa
    shift = shift.copy()
    shift[0] *= sa
    shift[1] *= sa

    # stream s=0 (y->o1): q=conv1, k=conv2, v=conv4
    # stream s=1 (x->o2): q=conv0, k=conv3, v=conv5
    qk_ids = [[1, 2], [0, 3]]
    v_ids = [4, 5]

    # wqk[s, qk, h, ci_chunk, ci, tap, co] = wf[conv, h, co, ci_glob, dy, dx]
    wqk = np.empty([2, 2, HEADS, C // P, P, 9, C], np.float32)
    wv = np.empty([2, HEADS, C // P, P, 9, C], np.float32)
    for s in range(2):
        for j, conv in enumerate(qk_ids[s]):
            # [h, co, ci, tap] -> [h, ci, tap, co]
            t = wf[conv].reshape(HEADS, C, C, 9).transpose(0, 2, 3, 1)
            wqk[s, j] = t.reshape(HEADS, C // P, P, 9, C)
        t = wf[v_ids[s]].reshape(HEADS, C, C, 9).transpose(0, 2, 3, 1)
        wv[s] = t.reshape(HEADS, C // P, P, 9, C)

    # bqk[s, qk, 128, h, co] = shift[conv][h, co] / 128 (replicated)
    bqk = np.empty([2, 2, P, HEADS, C], np.float32)
    for s in range(2):
        for j, conv in enumerate(qk_ids[s]):
            bqk[s, j] = np.broadcast_to(shift[conv][None], (P, HEADS, C)) / P

    # bv[s, h, dchunk, 128]
    bv = np.empty([2, HEADS, C // P, P], np.float32)
    for s in range(2):
        bv[s] = shift[v_ids[s]].reshape(HEADS, C // P, P)

    # wo[s, cchunk, c, co] = w_out.T / heads
    wo = np.empty([2, C // P, P, C], np.float32)
    wo[0] = (np.asarray(w_out1, np.float32).T / HEADS).reshape(C // P, P, C)
    wo[1] = (np.asarray(w_out2, np.float32).T / HEADS).reshape(C // P, P, C)

    return wqk, wv, bqk, bv, wo


def kernel(x, y, w_conv, bn_gamma, bn_beta, bn_mean, bn_var, w_out1, w_out2,
           h, w):
    H, W = int(h), int(w)
    x = np.asarray(x, np.float32)
    y = np.asarray(y, np.float32)
    B = x.shape[0]
    assert B == NCORES, f"expected B={NCORES}, got {B}"

    key = (H, W)
    if key not in _programs:
        _programs[key] = _build_program(H, W)
    nc = _programs[key]

    wqk, wv, bqk, bv, wo = _prep_weights(
        w_conv, bn_gamma, bn_beta, bn_mean, bn_var, w_out1, w_out2)

    in_maps = []
    for b in range(B):
        in_maps.append({
            "in0": np.ascontiguousarray(y[b]),   # stream 0: y -> o1
            "in1": np.ascontiguousarray(x[b]),   # stream 1: x -> o2
            "wqk": wqk, "wv": wv, "bqk": bqk, "bv": bv, "wo": wo,
        })

    trace = bool(int(os.environ.get("KERNEL_TRACE", "0")))
    res = run_bass_kernel_spmd(nc, in_maps, core_ids=list(range(NCORES)),
                               trace=trace)
    if trace:
        tr = res.instructions_and_trace
        print(f"[kernel] HW exec_time_ns={res.exec_time_ns} "
              f"mean={res.mean_exec_time_ns} "
              f"trace={tr[1] if tr else None}")
        kernel.last_exec_ns = res.exec_time_ns
        kernel.last_result = res

    o1 = np.stack([res.results[b]["out0"] for b in range(B)])
    o2 = np.stack([res.results[b]["out1"] for b in range(B)])
    return o1, o2
